# revision 19
# baseline (speedup 1.0000x reference)
"""GAT (2-layer, 4-head then 1-head) Bass kernel for TRN2, 8-way graph-parallel.

v2 — gather-paced design. Per core (cores own contiguous dst-node shards):
  - build1 (deduplicated): each core computes table rows only for ITS node
    shard (h1 | a_s | a_d packed in 512B rows), AllGather replicates the full
    table to every core. 8x less build work than replicated builds.
  - aggregation: edges sorted by (group of 2 dst-windows, table-half, window,
    src-row); group-major 128-edge slots (window straddles handled by per-slot
    "piece" lists). dma_gather pulls h[src] rows per slot; self-loop edges are
    NOT gathered (their rows are the core's own contiguous rows -> plain DMA
    into dedicated slots whose one-hot is the identity). Trailing padding uses
    idx=-1 which the gather ucode trims for free.
  - per slot: one-hot matrices from dst-locals via tensor_scalar(is_equal);
    a_d[dst] per edge via ohT^T @ adw matmul; e = Lrelu(a_s+a_d) on ACT;
    exp EXPANDED to all F columns on ACT so the alpha multiply is one clean
    unit-stride DVE op; scatter-add via oh^T @ msg into PSUM per window.
  - layer-1 drain feeds build2 (own shard) directly from SBUF; AllGather
    table2; layer 2 runs the same machinery (H=1, 256B rows).
Output: per-core dst shard [Nc, 64] fp32; host concatenates and adds b2.
"""

import math
import os
from contextlib import ExitStack

import numpy as np
import ml_dtypes

import concourse.bass as bass
import concourse.mybir as mybir
import concourse.tile as tile

P = 128
FP32 = mybir.dt.float32
BF16 = mybir.dt.bfloat16
I16 = mybir.dt.int16
AF = mybir.ActivationFunctionType
OP = mybir.AluOpType

NEG_SLOPE = 0.2
N_FULL = 50000
N_CORES = 8
GW = 2  # windows per group


# ----------------------------------------------------------------------------
# Host-side planning (pure index/structure work; no tensor-value compute)
# ----------------------------------------------------------------------------

class O:
    pass


def make_plan(edge_index: np.ndarray, N: int, n_cores: int):
    p = O()
    assert N % n_cores == 0
    Nc = N // n_cores                    # 6250
    n_win = math.ceil(Nc / P)            # 49
    NcP = n_win * P                      # 6272 (padded shard rows)
    NR = n_cores * NcP                   # 50176 (padded table rows)
    split = NR // 2                      # 25088, multiple of 128
    assert split % P == 0 and split < 32768 and (NR - split) < 32768

    src = edge_index[0].astype(np.int64)
    dst = edge_index[1].astype(np.int64)

    core = dst // Nc
    dl = dst % Nc
    win = dl // P
    grp = win // GW
    gl = dl - grp * GW * P               # group-local dst in [0, GW*128)
    row = (src // Nc) * NcP + (src % Nc)  # padded table row id
    half = (row >= split).astype(np.int64)
    rowh = row - half * split            # idx value (fits int16)

    n_grp = math.ceil(n_win / GW)
    # sort by (core, group, half, window, row)
    order = np.lexsort((row, win, half, grp, core))
    s_core, s_grp, s_half, s_gl, s_rowh = (
        core[order], grp[order], half[order], gl[order], rowh[order])

    counts = np.zeros((n_cores, n_grp, 2), dtype=np.int64)
    np.add.at(counts, (s_core, s_grp, s_half), 1)
    n_slots_gh = np.ceil(counts / P).astype(np.int64).max(axis=0)  # [n_grp, 2]

    # per (core, grp, half) start offsets in sorted arrays
    starts = np.zeros((n_cores, n_grp, 2), dtype=np.int64)
    pos = 0
    for c in range(n_cores):
        for g in range(n_grp):
            for h in range(2):
                starts[c, g, h] = pos
                pos += counts[c, g, h]
    assert pos == len(src)

    # also per (core, grp, half, win) counts for piece computation
    cw = np.zeros((n_cores, n_grp, 2, GW), dtype=np.int64)
    wk = win[order] - grp[order] * GW
    np.add.at(cw, (s_core, s_grp, s_half, wk), 1)

    groups = []
    slot0 = 0
    col0 = 0
    for g in range(n_grp):
        gi = O()
        gi.windows = list(range(g * GW, min((g + 1) * GW, n_win)))
        gi.g = g
        gi.nlo = int(n_slots_gh[g, 0])
        gi.nhi = int(n_slots_gh[g, 1])
        gi.n_self = len(gi.windows)
        gi.n_slots = gi.nlo + gi.nhi + gi.n_self
        gi.slot0 = slot0
        gi.lo_col0 = col0
        gi.hi_col0 = col0 + gi.nlo * (P // 16)
        gi.self_slots = [gi.nlo + gi.nhi + k for k in range(gi.n_self)]
        # memset ranges (slots that may contain junk tails on some core):
        # per half, from min_core(count)//P to n_slots of that half
        gi.memset = []
        for h, (base, nsl) in ((0, (0, gi.nlo)), (1, (gi.nlo, gi.nhi))):
            k0 = int(counts[:, g, h].min()) // P
            if k0 < nsl:
                gi.memset.append((base + k0, base + nsl))
        # pieces per gathered slot: union over cores of windows present
        gi.pieces = []
        for h, (base, nsl) in ((0, (0, gi.nlo)), (1, (gi.nlo, gi.nhi))):
            for i in range(nsl):
                ks = set()
                for c in range(n_cores):
                    # core c's edges in this (g,h) slot i: [i*P, (i+1)*P)
                    # window-k edges occupy [sum(cw[..k']<k), +cw[..k])
                    acc = 0
                    for k in range(len(gi.windows)):
                        a, b = acc, acc + int(cw[c, g, h, k])
                        acc = b
                        if a < (i + 1) * P and b > i * P:
                            ks.add(k)
                gi.pieces.append(sorted(ks))
        for k in range(gi.n_self):
            gi.pieces.append([k])
        col0 += (gi.nlo + gi.nhi) * (P // 16)
        slot0 += gi.n_slots
        groups.append(gi)

    S = slot0
    TOTC = col0  # idx tensor cols (of 16-wrapped)

    idx = np.zeros((n_cores, 16, TOTC), dtype=np.int16)
    dstloc = np.full((n_cores, P, S), -1.0, dtype=np.float32)

    for c in range(n_cores):
        for gi in groups:
            g = gi.g
            for h, (base, nsl, ccol0) in (
                    (0, (0, gi.nlo, gi.lo_col0)), (1, (gi.nlo, gi.nhi, gi.hi_col0))):
                cnt = int(counts[c, g, h])
                b0 = int(starts[c, g, h])
                padv = -1 if os.environ.get("GAT_PADNEG") else 0
                vals = np.full((nsl * P,), padv, dtype=np.int64)
                vals[:cnt] = s_rowh[b0:b0 + cnt]
                idx[c, :, ccol0:ccol0 + nsl * (P // 16)] = (
                    vals.astype(np.int16).reshape(nsl * P // 16, 16).T)
                dv = np.full((nsl * P,), -1.0, dtype=np.float32)
                dv[:cnt] = s_gl[b0:b0 + cnt]
                dstloc[c, :, gi.slot0 + base:gi.slot0 + base + nsl] = (
                    dv.reshape(nsl, P).T)
            for k, w in enumerate(gi.windows):
                nd = min(P, Nc - w * P)
                dv = np.full((P,), -1.0, dtype=np.float32)
                dv[:nd] = k * P + np.arange(nd)
                dstloc[c, :, gi.slot0 + gi.self_slots[k]] = dv

    p.N, p.n_cores, p.Nc, p.n_win, p.NcP, p.NR, p.split = (
        N, n_cores, Nc, n_win, NcP, NR, split)
    p.groups, p.S, p.TOTC, p.n_grp = groups, S, TOTC, n_grp
    p.Smax = max(gi.n_slots for gi in groups)
    p.idx = np.tile(idx, (1, 8, 1))      # replicate for 8 Q7 cores -> [*,128,*]
    p.dstloc = dstloc
    p.win_ndst = [min(P, Nc - w * P) for w in range(n_win)]
    return p


# ----------------------------------------------------------------------------
# Device program emitter
# ----------------------------------------------------------------------------

def emit_gat(tc, outs, ins, plan):
    nc = tc.nc
    DIS = set(filter(None, os.environ.get("GAT_DIS", "").split(",")))
    Nc, n_win, NcP, NR, split = plan.Nc, plan.n_win, plan.NcP, plan.NR, plan.split
    n_cores = plan.n_cores
    Smax = plan.Smax

    xT_own = ins["xT_own"]    # [128, NcP] bf16
    W1aug = ins["W1aug"]      # [128, 192] bf16 = [W1 | m1s | m1d]
    W2aug = ins["W2aug"]      # [128, 96]  bf16 = [W2 | m2s | m2d | pad]
    iota_in = ins["iota"]     # [128, 2*128] bf16 (col j of half k = j + 128k)
    ident_in = ins["ident"]   # [128, 128] bf16 identity
    idx_in = ins["idx"]       # [128, TOTC] i16
    dstloc_in = ins["dstloc"]   # [128, S] bf16
    dstlocF_in = ins["dstlocF"]  # [16, S*128] bf16 (free-major, 16x rep)
    out2 = outs["out2"]       # [Nc, 64] fp32

    ctx = ExitStack()
    with ctx:
        dram = ctx.enter_context(tc.tile_pool(name="dram", bufs=1, space="DRAM"))
        cpool = ctx.enter_context(tc.tile_pool(name="consts", bufs=1))
        bpool = ctx.enter_context(tc.tile_pool(name="build", bufs=3))
        bps = ctx.enter_context(tc.tile_pool(name="bps", bufs=2, space="PSUM"))
        rpool = ctx.enter_context(tc.tile_pool(name="rowsp", bufs=3))
        fpool = ctx.enter_context(tc.tile_pool(name="dstfp", bufs=2))
        spool = ctx.enter_context(tc.tile_pool(name="scorep", bufs=2))
        xpool = ctx.enter_context(tc.tile_pool(name="expp", bufs=2))
        ohpool = ctx.enter_context(tc.tile_pool(name="ohp", bufs=3))
        apsum = ctx.enter_context(tc.tile_pool(name="adops", bufs=2, space="PSUM"))
        wps = ctx.enter_context(tc.tile_pool(name="wps", bufs=3, space="PSUM"))
        tps = ctx.enter_context(tc.tile_pool(name="tps", bufs=1, space="PSUM"))
        dpool = ctx.enter_context(tc.tile_pool(name="drainp", bufs=2))

        tab1_own = dram.tile([NcP, 256], BF16, name="tab1_own")
        table1 = dram.tile([NR, 256], BF16, name="table1", addr_space="Shared")
        tab2_own = dram.tile([NcP, 128], BF16, name="tab2_own")
        table2 = dram.tile([NR, 128], BF16, name="table2", addr_space="Shared")

        # ---- constants to SBUF
        w1_sb = cpool.tile([P, 192], BF16, name="w1_sb")
        nc.sync.dma_start(out=w1_sb[:], in_=W1aug[:])
        w2_sb = cpool.tile([P, 96], BF16, name="w2_sb")
        nc.sync.dma_start(out=w2_sb[:], in_=W2aug[:])
        iota_sb = cpool.tile([P, 2, P], BF16, name="iota_sb")
        nc.sync.dma_start(out=iota_sb[:], in_=iota_in[:])
        iotaP_sb = cpool.tile([P, 2, P], BF16, name="iotaP_sb")
        nc.sync.dma_start(out=iotaP_sb[:], in_=ins["iotaP"][:])
        ident_sb = cpool.tile([P, P], BF16, name="ident_sb")
        nc.sync.dma_start(out=ident_sb[:], in_=ident_in[:])
        idx_sb = cpool.tile([P, plan.TOTC], I16, name="idx_sb")
        nc.sync.dma_start(out=idx_sb[:], in_=idx_in[:])
        dstl_sb = cpool.tile([P, plan.S], FP32, name="dstl_sb")
        nc.sync.dma_start(out=dstl_sb[:], in_=dstloc_in[:])
        adw1_sb = cpool.tile([P, n_win, 4], BF16, name="adw1_sb")
        adw2_sb = cpool.tile([P, n_win, 1], BF16, name="adw2_sb")
        o1T_sb = cpool.tile([P, NcP], BF16, name="o1T_sb")

        # ---- build1 (own shard only): rows [h1 bf16 x128 | a_s,a_d fp32 | pad]
        for b in range(n_win):
            xt = bpool.tile([P, P], BF16, name="xt", tag="xt")
            nc.sync.dma_start(out=xt[:], in_=xT_own[:, b * P:(b + 1) * P])
            ps = bps.tile([P, 192], FP32, name="psb", tag="psb")
            nc.tensor.matmul(out=ps[:], lhsT=xt[:], rhs=w1_sb[:],
                             start=True, stop=True)
            t1 = bpool.tile([P, 256], BF16, name="t1", tag="t1")
            nc.scalar.activation(out=t1[:, 0:128], in_=ps[:, 0:128], func=AF.Copy)
            t1f = t1[:].bitcast(FP32)
            nc.vector.tensor_copy(out=t1f[:, 64:72], in_=ps[:, 128:136])
            nc.vector.tensor_copy(out=adw1_sb[:, b, :], in_=ps[:, 132:136])
            eng = nc.sync if "sdma" in DIS else nc.scalar
            eng.dma_start(out=tab1_own[b * P:(b + 1) * P, :], in_=t1[:])

        stop = os.environ.get("GAT_STOP", "")
        if stop == "build1":
            nc.gpsimd.dma_start(out=out2[:, :],
                                in_=tab1_own[0:Nc, 0:128].bitcast(FP32))
            return

        nc.gpsimd.collective_compute(
            "AllGather", OP.bypass,
            replica_groups=[list(range(n_cores))],
            ins=[tab1_own[:]],
            outs=[table1[:]],
        )
        if stop == "ag1":
            nc.gpsimd.dma_start(out=out2[:, :],
                                in_=table1[0:Nc, 0:128].bitcast(FP32))
            return

        def emit_layer(layer):
            H = 4 if layer == 1 else 1
            F = 128 if layer == 1 else 64
            ROW = 256 if layer == 1 else 128   # table row elems (bf16)
            ASF = 64 if layer == 1 else 32     # fp32 col of embedded a_s
            tab = table1 if layer == 1 else table2
            tab_own = tab1_own if layer == 1 else tab2_own
            adw_sb = adw1_sb if layer == 1 else adw2_sb

            for gi in plan.groups:
                Sg = gi.n_slots
                rows = rpool.tile([P, Sg, ROW], BF16, name="rows",
                                  tag=f"rows{layer}",
                                  padded_shape=[P, Smax, ROW])
                # zero slots that may keep junk tails (before gather overwrite)
                if "memset" in DIS:
                    for (a, b) in gi.memset:
                        nc.vector.memset(rows[:, a:b, :], 0.0)
                # self-loop slots: own contiguous rows, no gather
                if "noself" not in DIS:
                    for k, w in enumerate(gi.windows):
                        nc.sync.dma_start(
                            out=rows[:, gi.self_slots[k], :],
                            in_=tab_own[w * P:(w + 1) * P, :])
                if gi.nlo:
                    nc.gpsimd.dma_gather(
                        out_ap=rows[:, 0:gi.nlo, :],
                        in_ap=tab[0:split, :],
                        idxs_ap=idx_sb[:, gi.lo_col0:gi.lo_col0 + gi.nlo * (P // 16)],
                        num_idxs=gi.nlo * P,
                        num_idxs_reg=gi.nlo * P,
                        elem_size=ROW,
                        single_packet=False,
                    )
                if gi.nhi:
                    nc.gpsimd.dma_gather(
                        out_ap=rows[:, gi.nlo:gi.nlo + gi.nhi, :],
                        in_ap=tab[split:NR, :],
                        idxs_ap=idx_sb[:, gi.hi_col0:gi.hi_col0 + gi.nhi * (P // 16)],
                        num_idxs=gi.nhi * P,
                        num_idxs_reg=gi.nhi * P,
                        elem_size=ROW,
                        single_packet=False,
                    )
                # free-major dst-locals broadcast to 128 partitions
                dstF = fpool.tile([P, Sg * P], BF16, name="dstF", tag="dstF",
                                  padded_shape=[P, Smax * P])
                nc.sync.dma_start(out=dstF[0:16, :],
                                  in_=dstlocF_in[:, gi.slot0 * P:(gi.slot0 + Sg) * P])
                r = 16
                while r < P:
                    nc.sync.dma_start(out=dstF[r:2 * r, :], in_=dstF[0:r, :])
                    r *= 2
                # edge-partition dst-locals replicated along free (for oh)
                dstlE = fpool.tile([P, Sg, P], BF16, name="dstlE", tag="dstlE",
                                   padded_shape=[P, Smax, P])
                nc.scalar.activation(
                    out=dstlE[:],
                    in_=dstl_sb[:, gi.slot0:gi.slot0 + Sg, None].to_broadcast(
                        [P, Sg, P]),
                    func=AF.Copy)
                # per-edge a_d via ohT matmuls into a group psum
                ado = apsum.tile([P, Sg, H], FP32, name="ado", tag="ado",
                                 padded_shape=[P, Smax, 4])
                if "ado" not in DIS:
                    for s in range(Sg):
                        ks = gi.pieces[s]
                        for j, k in enumerate(ks):
                            ohT = ohpool.tile([P, P], BF16, name="ohT", tag="ohT")
                            nc.vector.tensor_tensor(
                                out=ohT[:],
                                in0=iotaP_sb[:, k, :],
                                in1=dstF[:, s * P:(s + 1) * P],
                                op=OP.is_equal)
                            nc.tensor.matmul(
                                out=ado[:, s, :], lhsT=ohT[:],
                                rhs=adw_sb[:, gi.windows[k], :],
                                start=(j == 0), stop=(j == len(ks) - 1))
                # e = a_s[src] + a_d[dst]; Lrelu on ACT; exp expanded to F
                rows_f = rows[:].bitcast(FP32)     # [P, Sg, ROW//2]
                e_t = spool.tile([P, Sg, H], FP32, name="e_t", tag="e_t",
                                 padded_shape=[P, Smax, 4])
                if "ado" in DIS:
                    nc.vector.tensor_copy(out=e_t[:], in_=rows_f[:, :, ASF:ASF + H])
                else:
                    nc.vector.tensor_tensor(out=e_t[:], in0=rows_f[:, :, ASF:ASF + H],
                                            in1=ado[:], op=OP.add)
                l_t = spool.tile([P, Sg, H], FP32, name="l_t", tag="l_t",
                                 padded_shape=[P, Smax, 4])
                nc.vector.tensor_scalar_mul(out=l_t[:], in0=e_t[:],
                                            scalar1=NEG_SLOPE)
                nc.vector.tensor_tensor(out=l_t[:], in0=e_t[:], in1=l_t[:],
                                        op=OP.max)
                expF = xpool.tile([P, Sg, H, F // H], BF16, name="expF",
                                  tag=f"expF{layer}",
                                  padded_shape=[P, Smax, H, F // H])
                expS = xpool.tile([P, Sg, H], BF16, name="expS", tag="expS",
                                  padded_shape=[P, Smax, 4])
                nc.scalar.activation(out=expS[:], in_=l_t[:], func=AF.Exp)
                if "expact" not in DIS:
                    nc.scalar.activation(
                        out=expF[:],
                        in_=l_t[:, :, :, None].to_broadcast([P, Sg, H, F // H]),
                        func=AF.Exp)
                # per-window scatter state
                psw = {}
                first = {}
                npc = [0] * len(gi.windows)
                for s in range(Sg):
                    for k in gi.pieces[s]:
                        npc[k] += 1
                for k, w in enumerate(gi.windows):
                    psw[k] = wps.tile([P, F + H], FP32, name="psw", tag="psw",
                                      padded_shape=[P, 132])
                    first[k] = True
                done = [0] * len(gi.windows)
                if "scat" not in DIS:
                    # in-place: rows[:, :, 0:F] *= expF ; denom at cols F:F+H
                    nc.vector.tensor_tensor(
                        out=rows[:, :, 0:F], in0=rows[:, :, 0:F],
                        in1=expF[:, :, :, :], op=OP.mult)
                    nc.scalar.activation(out=rows[:, :, F:F + H], in_=expS[:],
                                         func=AF.Copy)
                for s in range(Sg if "scat" not in DIS else 0):
                    for k in gi.pieces[s]:
                        oh = ohpool.tile([P, P], BF16, name="oh", tag="oh")
                        nc.vector.tensor_tensor(
                            out=oh[:], in0=iota_sb[:, k, :],
                            in1=dstlE[:, s, :], op=OP.is_equal)
                        done[k] += 1
                        nc.tensor.matmul(out=psw[k][:], lhsT=oh[:],
                                         rhs=rows[:, s, 0:F + H],
                                         start=first[k], stop=(done[k] == npc[k]))
                        first[k] = False
                # drain windows
                for k, w in enumerate(gi.windows if "drain" not in DIS else []):
                    Dw = plan.win_ndst[w]
                    den = dpool.tile([P, H], FP32, name="den", tag="den",
                                     padded_shape=[P, 4])
                    nc.vector.tensor_scalar_add(out=den[:], in0=psw[k][:, F:F + H],
                                                scalar1=1e-16)
                    rec = dpool.tile([P, H], FP32, name="rec", tag="rec",
                                     padded_shape=[P, 4])
                    nc.vector.reciprocal(out=rec[:], in_=den[:])
                    if layer == 1:
                        o1 = dpool.tile([P, 128], FP32, name="o1", tag="o1")
                        for h in range(H):
                            nc.vector.tensor_scalar(
                                out=o1[:, h * 32:(h + 1) * 32],
                                in0=psw[k][:, h * 32:(h + 1) * 32],
                                scalar1=rec[:, h:h + 1], scalar2=None, op0=OP.mult)
                        o1b = dpool.tile([P, 128], BF16, name="o1b", tag="o1b")
                        nc.vector.tensor_scalar_max(out=o1b[:], in0=o1[:], scalar1=0.0)
                        pst = tps.tile([P, P], BF16, name="pst", tag="pst")
                        nc.tensor.transpose(out=pst[:], in_=o1b[:], identity=ident_sb[:])
                        if "actcopy" in DIS:
                            nc.vector.tensor_copy(out=o1T_sb[:, w * P:(w + 1) * P],
                                                  in_=pst[:])
                        else:
                            nc.scalar.activation(out=o1T_sb[:, w * P:(w + 1) * P],
                                                 in_=pst[:], func=AF.Copy)
                        # build2 for this window, straight from SBUF
                        ps2 = bps.tile([P, 96], FP32, name="ps2", tag="psb")
                        nc.tensor.matmul(out=ps2[:], lhsT=o1T_sb[:, w * P:(w + 1) * P],
                                         rhs=w2_sb[:], start=True, stop=True)
                        t2 = bpool.tile([P, 128], BF16, name="t2", tag="t1")
                        nc.scalar.activation(out=t2[:, 0:64], in_=ps2[:, 0:64],
                                             func=AF.Copy)
                        t2f = t2[:].bitcast(FP32)
                        nc.vector.tensor_copy(out=t2f[:, 32:34], in_=ps2[:, 64:66])
                        nc.vector.tensor_copy(out=adw2_sb[:, w, :], in_=ps2[:, 65:66])
                        eng2 = nc.sync if "sdma" in DIS else nc.scalar
                        eng2.dma_start(out=tab2_own[w * P:(w + 1) * P, :], in_=t2[:])
                    else:
                        o2 = dpool.tile([P, 64], FP32, name="o2", tag="o2")
                        nc.vector.tensor_scalar(out=o2[:], in0=psw[k][:, 0:64],
                                                scalar1=rec[:, 0:1], scalar2=None,
                                                op0=OP.mult)
                        nc.sync.dma_start(out=out2[w * P:w * P + Dw, :],
                                          in_=o2[:Dw, :])

        emit_layer(1)
        if stop == "l1":
            nc.gpsimd.dma_start(out=out2[:, :],
                                in_=tab2_own[0:Nc, 0:128].bitcast(FP32))
            return

        nc.gpsimd.collective_compute(
            "AllGather", OP.bypass,
            replica_groups=[list(range(n_cores))],
            ins=[tab2_own[:]],
            outs=[table2[:]],
        )
        if stop == "ag2":
            nc.gpsimd.dma_start(out=out2[:, :],
                                in_=table2[0:Nc, 0:128].bitcast(FP32))
            return

        emit_layer(2)


# ----------------------------------------------------------------------------
# Host input construction
# ----------------------------------------------------------------------------

def build_host_inputs(plan, x, W1, att_src1, att_dst1, W2, att_src2, att_dst2):
    bf = ml_dtypes.bfloat16
    HID = 32
    H1 = att_src1.shape[0]
    m1s = np.stack([W1[:, h * HID:(h + 1) * HID] @ att_src1[h] for h in range(H1)], axis=1)
    m1d = np.stack([W1[:, h * HID:(h + 1) * HID] @ att_dst1[h] for h in range(H1)], axis=1)
    m2s = (W2 @ att_src2[0])[:, None]
    m2d = (W2 @ att_dst2[0])[:, None]
    W1aug = np.zeros((128, 192), np.float32)
    W1aug[:, 0:128] = W1
    W1aug[:, 128:132] = m1s
    W1aug[:, 132:136] = m1d
    W1aug = W1aug.astype(bf)
    W2aug = np.zeros((128, 96), np.float32)
    W2aug[:, :64] = W2
    W2aug[:, 64:65] = m2s
    W2aug[:, 65:66] = m2d
    W2aug = W2aug.astype(bf)

    xT = np.ascontiguousarray(x.T).astype(bf)  # [128, N]
    iota = np.zeros((128, 2, 128), np.float32)
    iota[:, 0, :] = np.arange(128)[None, :]
    iota[:, 1, :] = 128 + np.arange(128)[None, :]
    iota = iota.reshape(128, 256).astype(bf)
    iotaP = np.zeros((128, 2, 128), np.float32)
    iotaP[:, 0, :] = np.arange(128)[:, None]
    iotaP[:, 1, :] = 128 + np.arange(128)[:, None]
    iotaP = iotaP.reshape(128, 256).astype(bf)
    ident = np.eye(128, dtype=np.float32).astype(bf)

    shared = dict(W1aug=W1aug, W2aug=W2aug, iota=iota, iotaP=iotaP, ident=ident)
    in_maps = []
    for c in range(plan.n_cores):
        m = dict(shared)
        xo = np.zeros((128, plan.NcP), np.float32)
        xo[:, :plan.Nc] = xT[:, c * plan.Nc:(c + 1) * plan.Nc].astype(np.float32)
        m["xT_own"] = xo.astype(bf)
        m["idx"] = plan.idx[c]
        m["dstloc"] = np.asarray(plan.dstloc[c], np.float32)
        m["dstlocF"] = np.tile(np.ascontiguousarray(
            plan.dstloc[c].astype(ml_dtypes.bfloat16).T).reshape(1, -1), (16, 1))
        in_maps.append(m)
    return in_maps


# ----------------------------------------------------------------------------
# Harness entry point
# ----------------------------------------------------------------------------

LAST_RESULT = None


def _ensure_ntff_hook():
    import sys
    import types
    try:
        import antenv.axon_hooks  # noqa: F401
        return
    except ImportError:
        pass
    mod = types.ModuleType("antenv.axon_hooks")
    state = {}
    mod.set_axon_ntff_profile_hook = lambda h: state.__setitem__("h", h)
    mod.get_axon_ntff_profile_hook = lambda: state.get("h")
    import antenv
    sys.modules["antenv.axon_hooks"] = mod
    antenv.axon_hooks = mod
    try:
        from trn_agent_boot.trn_boot import _ntff_profile_via_ctypes
        hook = _ntff_profile_via_ctypes("/opt/axon/libaxon_pjrt.so")
        if hook is not None:
            mod.set_axon_ntff_profile_hook(hook)
    except Exception as e:  # noqa: BLE001
        print("ntff hook setup failed:", e)


def _build_nc(plan):
    import concourse.bacc as bacc
    nc = bacc.Bacc("TRN2", target_bir_lowering=False, debug=False,
                   num_devices=plan.n_cores)
    ins_t = {
        "xT_own": nc.dram_tensor("xT_own", [128, plan.NcP], BF16,
                                 kind="ExternalInput").ap(),
        "W1aug": nc.dram_tensor("W1aug", [128, 192], BF16, kind="ExternalInput").ap(),
        "W2aug": nc.dram_tensor("W2aug", [128, 96], BF16, kind="ExternalInput").ap(),
        "iota": nc.dram_tensor("iota", [128, 256], BF16, kind="ExternalInput").ap(),
        "iotaP": nc.dram_tensor("iotaP", [128, 256], BF16, kind="ExternalInput").ap(),
        "ident": nc.dram_tensor("ident", [128, 128], BF16, kind="ExternalInput").ap(),
        "idx": nc.dram_tensor("idx", [128, plan.TOTC], I16,
                              kind="ExternalInput").ap(),
        "dstloc": nc.dram_tensor("dstloc", [128, plan.S], FP32,
                                 kind="ExternalInput").ap(),
        "dstlocF": nc.dram_tensor("dstlocF", [16, plan.S * 128], BF16,
                                  kind="ExternalInput").ap(),
    }
    outs_t = {
        "out2": nc.dram_tensor("out2", [plan.Nc, 64], FP32,
                               kind="ExternalOutput").ap(),
    }
    with tile.TileContext(nc) as t:
        emit_gat(t, outs_t, ins_t, plan)
    nc.compile()
    return nc


def kernel(**inputs):
    global LAST_RESULT
    from concourse.bass_utils import run_bass_kernel_spmd

    x = np.asarray(inputs["x"], np.float32)
    edge_index = np.asarray(inputs["edge_index"])
    W1 = np.asarray(inputs["W1"], np.float32)
    as1 = np.asarray(inputs["att_src1"], np.float32)
    ad1 = np.asarray(inputs["att_dst1"], np.float32)
    b1 = np.asarray(inputs["b1"], np.float32)
    W2 = np.asarray(inputs["W2"], np.float32)
    as2 = np.asarray(inputs["att_src2"], np.float32)
    ad2 = np.asarray(inputs["att_dst2"], np.float32)
    b2 = np.asarray(inputs["b2"], np.float32)
    assert float(np.abs(b1).max()) == 0.0, "nonzero b1 not supported"

    N = x.shape[0]
    plan = make_plan(edge_index, N, N_CORES)
    in_maps = build_host_inputs(plan, x, W1, as1, ad1, W2, as2, ad2)
    nc = _build_nc(plan)
    trace = os.environ.get("GAT_TRACE", "0") == "1"
    if trace:
        _ensure_ntff_hook()
    res = run_bass_kernel_spmd(nc, in_maps, core_ids=list(range(plan.n_cores)),
                               trace=trace)
    LAST_RESULT = res
    out = np.concatenate([res.results[c]["out2"] for c in range(plan.n_cores)],
                         axis=0)
    return (out + b2[None, :]).astype(np.float32)
